# Initial kernel scaffold
#
"""Fused MergedQKVParallelLinearWithDelta kernel for 8 Trainium2 NeuronCores.

Strategy (tensor-parallel on the QKV output dim, as in vLLM):
  - Each core owns a 768-row output shard (512 q + 128 k + 128 v rows).
  - Tokens are sorted by adapter index on the host; the device gathers token
    rows with an indirect DMA (f32->f16 cast in flight), permutes the K dim to
    nibble-extraction order, and DMA-transposes to K-major tiles.
  - GPTQ 4-bit delta weights are DMA-transposed as uint16, nibble-extracted on
    DVE (fused shift+and), and converted to f16 (DVE/GPSIMD).
  - All matmuls compute out^T [o, t] with weights stationary, so scales/zeros
    are per-partition scalars: out = psum_base + sc[o] * psum_delta where
    psum_delta = sum_k x*w4 - (z+1)*rowsum(x) (the z term is a rank-1 matmul).
  - The host de-permutes/reassembles the 8 transposed shards.
"""

import math
from contextlib import ExitStack

import numpy as np

import concourse.bass as bass
import concourse.tile as tile
from concourse import bacc
from concourse import mybir
from concourse.bass_utils import run_bass_kernel_spmd

N_CORES = 8
T, IN = 1024, 4096
Q, KV = 4096, 1024
OUT = Q + 2 * KV
D = 4
OS = OUT // N_CORES          # 768 output rows per core
NB = IN // 128               # 32 K' tiles
SLICE = 512
PACKW = IN // 4              # 1024 uint16 words per row

F16 = mybir.dt.float16
F32 = mybir.dt.float32
U16 = mybir.dt.uint16
I32 = mybir.dt.int32

# ---------------------------------------------------------------------------
# Host-side routing schedule
# ---------------------------------------------------------------------------
def _schedule(indices):
    idx = np.asarray(indices).astype(np.int64)
    tile_adapters = []
    gather_parts = []
    orig_parts = []
    for d in range(D):
        toks = np.nonzero(idx == d)[0]
        if len(toks) == 0:
            continue
        n_t = (len(toks) + 127) // 128
        pad = n_t * 128 - len(toks)
        gather_parts.append(np.concatenate([toks, np.zeros(pad, np.int64)]))
        orig_parts.append(np.concatenate([toks, -np.ones(pad, np.int64)]))
        tile_adapters += [d] * n_t
    gather = np.concatenate(gather_parts).astype(np.int32)
    origs = np.concatenate(orig_parts).astype(np.int64)
    return tuple(tile_adapters), gather, origs


def _slices_and_runs(tile_adapters):
    n_tiles = len(tile_adapters)
    t_pad = n_tiles * 128
    slices = []
    c = 0
    while c < t_pad:
        slices.append((c, min(c + SLICE, t_pad)))
        c += SLICE
    runs = []  # per slice: list of (col0_in_slice, ncols, adapter)
    for c0, c1 in slices:
        rr = []
        for i in range(c0 // 128, c1 // 128):
            d = tile_adapters[i]
            col = i * 128 - c0
            if rr and rr[-1][2] == d and rr[-1][0] + rr[-1][1] == col:
                rr[-1] = (rr[-1][0], rr[-1][1] + 128, d)
            else:
                rr.append((col, 128, d))
        runs.append(rr)
    return slices, runs


# ---------------------------------------------------------------------------
# Device program
# ---------------------------------------------------------------------------
DEBUG_TAPS = 0


def _build_program(tile_adapters, split_waits=True):
    n_tiles = len(tile_adapters)
    t_pad = n_tiles * 128
    slices, runs = _slices_and_runs(tile_adapters)
    n_s = len(slices)
    adapters_present = sorted(set(tile_adapters))

    nc = bacc.Bacc(
        trn_type="TRN2", target_bir_lowering=False, debug=False, num_devices=1
    )
    x_d = nc.dram_tensor("x", [T, IN], F32, kind="ExternalInput").ap()
    gidx_d = nc.dram_tensor("gidx", [t_pad, 1], I32, kind="ExternalInput").ap()
    wb_d = nc.dram_tensor("wb", [OS, IN], F32, kind="ExternalInput").ap()
    qwu_d = nc.dram_tensor("qwu", [D, OS, PACKW], U16, kind="ExternalInput").ap()
    biasr_d = nc.dram_tensor("biasr", [1, OS], F16, kind="ExternalInput").ap()
    znr_d = nc.dram_tensor("znr", [1, D * OS], F16, kind="ExternalInput").ap()
    scc_d = nc.dram_tensor("scc", [128, (OS // 128) * D], F32, kind="ExternalInput").ap()
    outT_d = nc.dram_tensor("outT", [OS, t_pad], F32, kind="ExternalOutput").ap()
    if DEBUG_TAPS in (2, 3, 4):
        wdump_d = nc.dram_tensor(
            "wdump", [OS // 128, NB, 128, 128 * (1 + D)], F16, kind="ExternalOutput"
        ).ap()
    if DEBUG_TAPS in (1, 3):
        xdump_d = nc.dram_tensor(
            "xdump", [NB, 128, t_pad], F16, kind="ExternalOutput"
        ).ap()
        udump_d = nc.dram_tensor(
            "udump", [2, t_pad], F16, kind="ExternalOutput"
        ).ap()

    with TileCtx(nc) as tc, ExitStack() as ctx:
        pmisc = ctx.enter_context(tc.tile_pool(name="misc", bufs=1))
        pgi = ctx.enter_context(tc.tile_pool(name="gi", bufs=2))
        pin = ctx.enter_context(tc.tile_pool(name="ain", bufs=2))
        pperm = ctx.enter_context(tc.tile_pool(name="perm", bufs=2))
        pxgT = ctx.enter_context(tc.tile_pool(name="xgT", bufs=1))
        pw = ctx.enter_context(tc.tile_pool(name="wpool", bufs=36))
        pqt = ctx.enter_context(tc.tile_pool(name="qt", bufs=6))
        pext = ctx.enter_context(tc.tile_pool(name="ext", bufs=6))
        pps = ctx.enter_context(tc.tile_pool(name="ps", bufs=1, space="PSUM"))
        pout = ctx.enter_context(tc.tile_pool(name="outp", bufs=4))

        # constants
        biasr = pmisc.tile([1, OS], F16, tag="biasr")
        nc.gpsimd.dma_start(biasr[:], biasr_d[:])
        znr = pmisc.tile([1, D * OS], F16, tag="znr")
        nc.gpsimd.dma_start(znr[:], znr_d[:])
        scc = pmisc.tile([128, (OS // 128) * D], F32, tag="scc")
        nc.gpsimd.dma_start(scc[:], scc_d[:])
        ones_col = pmisc.tile([128, 1], F16, tag="onesc")
        nc.vector.memset(ones_col[:], 1.0)
        ones_row = pmisc.tile([1, SLICE], F16, tag="onesr")
        nc.vector.memset(ones_row[:], 1.0)

        # xgT[kb][s] : [128, slen] f16  (K'-major gathered activations)
        xgT = [
            [
                pxgT.tile(
                    [128, c1 - c0], F16, tag=f"xgT_{kb}_{s}", name=f"xgT_{kb}_{s}"
                )
                for s, (c0, c1) in enumerate(slices)
            ]
            for kb in range(NB)
        ]

        def sigma_copy(dst, src):
            # dst[.., 512*C + 128*jj + p] = src[.., 512*C + 4*p + jj]
            sv = src.rearrange("a (b p j) -> a b j p", b=IN // 512, p=128, j=4)
            dv = dst.rearrange("a (b j p) -> a b j p", b=IN // 512, j=4, p=128)
            nc.scalar.copy(dv, sv)

        # ---- Phase A: gather + permute + transpose activations
        for i in range(n_tiles):
            s_i = (i * 128) // SLICE
            col = i * 128 - slices[s_i][0]
            gi = pgi.tile([128, 1], I32, tag="gi")
            nc.gpsimd.dma_start(gi[:], gidx_d[i * 128 : (i + 1) * 128, :])
            gx = pin.tile([128, IN], F16, tag="ain")
            nc.gpsimd.indirect_dma_start(
                out=gx[:],
                out_offset=None,
                in_=x_d[:],
                in_offset=bass.IndirectOffsetOnAxis(ap=gi[:, :1], axis=0),
            )
            px = pperm.tile([128, IN], F16, tag="perm")
            sigma_copy(px[:], gx[:])
            for kb in range(NB):
                nc.sync.dma_start(
                    xgT[kb][s_i][:, col : col + 128],
                    px[:, kb * 128 : (kb + 1) * 128],
                    transpose=True,
                )

        # ---- u rows: colsum of xg (fp16) per slice
        u_rows = []
        for s, (c0, c1) in enumerate(slices):
            slen = c1 - c0
            ups = pps.tile([1, slen], F32, space="PSUM", tag="ups")
            for kb in range(NB):
                nc.tensor.matmul(
                    ups[:],
                    lhsT=ones_col[:],
                    rhs=xgT[kb][s][:],
                    start=(kb == 0),
                    stop=(kb == NB - 1),
                )
            ur = pmisc.tile([1, slen], F16, tag=f"urow{s}")
            nc.vector.tensor_copy(ur[:], ups[:])
            url = pmisc.tile([1, slen], F16, tag=f"urowl{s}")
            nc.vector.tensor_tensor(
                out=url[:], in0=ups[:], in1=ur[:], op=mybir.AluOpType.subtract
            )
            u_rows.append((ur, url))
            if DEBUG_TAPS in (1, 3):
                nc.sync.dma_start(udump_d[0:1, c0:c1], ur[:])
                nc.sync.dma_start(udump_d[1:2, c0:c1], url[:])

        # ---- Phase B: per 128-row output tile
        conv_rr = 0
        for ot in range(OS // 128):
            o0 = 128 * ot
            orng = slice(o0, o0 + 128)

            # base weights: cast-load, sigma-permute, transpose into Wt[:, 0:128]
            wbt = pin.tile([128, IN], F16, tag="ain")
            nc.gpsimd.dma_start(wbt[:], wb_d[orng, :])
            wbp = pperm.tile([128, IN], F16, tag="perm")
            sigma_copy(wbp[:], wbt[:])
            wt = [pw.tile([128, 128 * (1 + D)], F16, tag="W", name=f"wt_{ot}_{k}") for k in range(NB)]
            for kb in range(NB):
                nc.sync.dma_start(
                    wt[kb][:, 0:128],
                    wbp[:, kb * 128 : (kb + 1) * 128],
                    transpose=True,
                )

            # delta weights: u16 transpose + nibble extract + convert
            for C in range(PACKW // 128):
                qt = pqt.tile([128, 128 * D], U16, tag="qt")
                for d in range(D):
                    nc.sync.dma_start(
                        qt[:, d * 128 : (d + 1) * 128],
                        qwu_d[d, orng, C * 128 : (C + 1) * 128],
                        transpose=True,
                    )
                for jj in range(4):
                    kb = 4 * C + jj
                    ex = pext.tile([128, 128 * D], U16, tag="ex")
                    if jj == 0:
                        nc.vector.tensor_scalar(
                            out=ex[:], in0=qt[:], scalar1=0xF, scalar2=None,
                            op0=mybir.AluOpType.bitwise_and,
                        )
                    elif jj == 3:
                        nc.vector.tensor_scalar(
                            out=ex[:], in0=qt[:], scalar1=12, scalar2=None,
                            op0=mybir.AluOpType.logical_shift_right,
                        )
                    else:
                        nc.vector.tensor_scalar(
                            out=ex[:], in0=qt[:], scalar1=4 * jj, scalar2=0xF,
                            op0=mybir.AluOpType.logical_shift_right,
                            op1=mybir.AluOpType.bitwise_and,
                        )
                    eng = nc.vector  # gpsimd convert suspected racy
                    eng.tensor_copy(wt[kb][:, 128 : 128 * (1 + D)], ex[:])
                    conv_rr += 1

            if DEBUG_TAPS in (2, 3):
                for kb in range(NB):
                    nc.sync.dma_start(wdump_d[ot, kb], wt[kb][:])
            if DEBUG_TAPS in (1, 3) and ot == 0:
                for kb in range(NB):
                    for s_, (c0_, c1_) in enumerate(slices):
                        nc.sync.dma_start(
                            xdump_d[kb, :, c0_:c1_], xgT[kb][s_][:]
                        )
            # matmuls: out^T accumulation
            psb = []
            psd = []
            for s, (c0, c1) in enumerate(slices):
                slen = c1 - c0
                b = pps.tile([128, slen], F32, space="PSUM", tag=f"psb{s}")
                dl = pps.tile([128, slen], F32, space="PSUM", tag=f"psd{s}")
                psb.append(b)
                psd.append(dl)
                nc.tensor.matmul(
                    b[:],
                    lhsT=biasr[0:1, orng],
                    rhs=ones_row[0:1, 0:slen],
                    start=True,
                    stop=False,
                )
            for kb in range(NB):
                for s in range(n_s):
                    nc.tensor.matmul(
                        psb[s][:],
                        lhsT=wt[kb][:, 0:128],
                        rhs=xgT[kb][s][:],
                        start=False,
                        stop=(kb == NB - 1),
                    )
                    for ri, (rc0, rn, d) in enumerate(runs[s]):
                        nc.tensor.matmul(
                            psd[s][:, rc0 : rc0 + rn],
                            lhsT=wt[kb][:, 128 * (1 + d) : 128 * (2 + d)],
                            rhs=xgT[kb][s][:, rc0 : rc0 + rn],
                            start=(kb == 0 and ri == 0),
                            stop=False,
                        )
            for s in range(n_s):
                for ri, (rc0, rn, d) in enumerate(runs[s]):
                    for ui, upart in enumerate(u_rows[s]):
                        nc.tensor.matmul(
                            psd[s][:, rc0 : rc0 + rn],
                            lhsT=znr[0:1, d * OS + o0 : d * OS + o0 + 128],
                            rhs=upart[0:1, rc0 : rc0 + rn],
                            start=False,
                            stop=(ri == len(runs[s]) - 1 and ui == 1),
                        )
                oo = pout.tile([128, slices[s][1] - slices[s][0]], F32, tag="o")
                tmp = pout.tile(
                    [128, slices[s][1] - slices[s][0]], F32, tag="otmp"
                )
                for rc0, rn, d in runs[s]:
                    nc.scalar.mul(
                        tmp[:, rc0 : rc0 + rn],
                        psd[s][:, rc0 : rc0 + rn],
                        scc[:, ot * D + d : ot * D + d + 1],
                    )
                nc.vector.tensor_tensor(
                    out=oo[:],
                    in0=tmp[:],
                    in1=psb[s][:],
                    op=mybir.AluOpType.add,
                )
                nc.gpsimd.dma_start(
                    outT_d[orng, slices[s][0] : slices[s][1]], oo[:]
                )

    if split_waits:
        nc.compile()
    return nc


def TileCtx(nc):
    return tile.TileContext(nc)


# ---------------------------------------------------------------------------
# Host wrapper
# ---------------------------------------------------------------------------
def _unpack_zeros(qz, o_count):
    # qz: [D, o_count//8, 1] int32; returns [D, o_count] float zeros
    o = np.arange(o_count)
    words = qz[:, o >> 3, 0].astype(np.int64)
    return ((words >> (4 * (o & 7))) & 0xF).astype(np.float32)


_prog_cache = {}


def kernel(**inputs):
    x = np.ascontiguousarray(np.asarray(inputs["x"], dtype=np.float32))
    w_base = np.asarray(inputs["w_base"], dtype=np.float32)
    bias = np.asarray(inputs["bias"], dtype=np.float32)
    qw_q = np.asarray(inputs["qweight_q"], dtype=np.int32)
    qw_k = np.asarray(inputs["qweight_k"], dtype=np.int32)
    qw_v = np.asarray(inputs["qweight_v"], dtype=np.int32)
    qz_q = np.asarray(inputs["qzeros_q"], dtype=np.int32)
    qz_k = np.asarray(inputs["qzeros_k"], dtype=np.int32)
    qz_v = np.asarray(inputs["qzeros_v"], dtype=np.int32)
    sc_q = np.asarray(inputs["scales_q"], dtype=np.float32)
    sc_k = np.asarray(inputs["scales_k"], dtype=np.float32)
    sc_v = np.asarray(inputs["scales_v"], dtype=np.float32)
    indices = np.asarray(inputs["indices"])

    tile_adapters, gather, origs = _schedule(indices)
    t_pad = len(tile_adapters) * 128

    if tile_adapters not in _prog_cache:
        _prog_cache[tile_adapters] = _build_program(tile_adapters)
    nc = _prog_cache[tile_adapters]

    z_q = _unpack_zeros(qz_q, Q)
    z_k = _unpack_zeros(qz_k, KV)
    z_v = _unpack_zeros(qz_v, KV)

    SQ, SK = Q // N_CORES, KV // N_CORES
    in_maps = []
    for c in range(N_CORES):
        qs = slice(SQ * c, SQ * (c + 1))
        ks = slice(SK * c, SK * (c + 1))
        wb = np.concatenate(
            [w_base[qs], w_base[Q + SK * c : Q + SK * (c + 1)],
             w_base[Q + KV + SK * c : Q + KV + SK * (c + 1)]], axis=0
        )
        qw = np.concatenate([qw_q[:, qs], qw_k[:, ks], qw_v[:, ks]], axis=1)
        qwu = np.ascontiguousarray(qw).view(np.uint16).reshape(D, OS, PACKW)
        z = np.concatenate([z_q[:, qs], z_k[:, ks], z_v[:, ks]], axis=1)
        sc = np.concatenate(
            [sc_q[:, qs, 0], sc_k[:, ks, 0], sc_v[:, ks, 0]], axis=1
        )
        b = np.concatenate(
            [bias[qs], bias[Q + SK * c : Q + SK * (c + 1)],
             bias[Q + KV + SK * c : Q + KV + SK * (c + 1)]]
        )
        znr = (-(z + 1.0)).astype(np.float16)
        biasr = np.ascontiguousarray(b.astype(np.float16)[None, :])
        scc = np.zeros([128, (OS // 128) * D], np.float32)
        for ot in range(OS // 128):
            for d in range(D):
                scc[:, ot * D + d] = sc[d, 128 * ot : 128 * (ot + 1)]
        in_maps.append(
            {
                "x": x,
                "gidx": np.ascontiguousarray(gather[:, None]),
                "wb": np.ascontiguousarray(wb),
                "qwu": qwu,
                "biasr": biasr,
                "znr": np.ascontiguousarray(znr.reshape(1, -1)),
                "scc": scc,
            }
        )

    import os

    trace = bool(int(os.environ.get("KERNEL_TRACE", "0")))
    res = run_bass_kernel_spmd(
        nc, in_maps, core_ids=list(range(N_CORES)), trace=trace
    )
    kernel._last_results = res

    out = np.zeros([T, OUT], np.float32)
    valid = origs >= 0
    vpos = np.nonzero(valid)[0]
    vtok = origs[valid]
    for c in range(N_CORES):
        rT = res.results[c]["outT"]  # [OS, t_pad]
        r = np.asarray(rT).T  # [t_pad, OS]
        cols = np.concatenate(
            [
                np.arange(SQ * c, SQ * (c + 1)),
                np.arange(Q + SK * c, Q + SK * (c + 1)),
                np.arange(Q + KV + SK * c, Q + KV + SK * (c + 1)),
            ]
        )
        out[vtok[:, None], cols[None, :]] = r[vpos]
    return out



# revision 22
# speedup vs baseline: 4.2813x; 4.2813x over previous
"""Fused MergedQKVParallelLinearWithDelta kernel for 8 Trainium2 NeuronCores.

Strategy (tensor-parallel on the QKV output dim, vLLM-style):
  - Each core owns a 768-row output shard (512 q + 128 k + 128 v rows).
  - Host pre-lays-out all weights K-major (no device-side weight
    transposes): w_base transposed to [IN, OS] f16; the GPTQ nibbles are
    repacked so that a single shift-and-mask extraction of plane jj from
    u16-word chunk C yields weights for the natural k-block 512C+128jj..+128.
  - Tokens are sorted by adapter on the host (no inter-adapter padding;
    T==1024 is already a multiple of 128).  The device gathers token rows
    with an indirect DMA (f32->f16 cast in flight) and transposes each
    128-token tile to K-major with ONE batched DMA-transpose instruction
    whose 3D output AP writes all 32 k-blocks at once.
  - Per adapter d the device builds merged weights
        wfull[k, o] = w_base[o, k] + sc[d, o] * w4[d, o, k]      (f16)
    (extract u16 on DVE, scale-multiply with partition-replicated scales
    that also converts to f16, base-add on GpSimd), so each token needs
    only ONE matmul pass over K.  Bias and the GPTQ zero-point correction
    -(z+1)*sc * colsum(x) enter as a single K=3 aux matmul per PSUM piece
    (rows: ones / u_hi / u_lo residual for f16 precision).
  - wfull is split into o-halves so building adapter d+1's half overlaps
    the matmuls of adapter d's other half (single-buffered tiles, WAR
    deps via the tile framework).
  - The host de-permutes/reassembles the 8 transposed output shards.
"""

import numpy as np

import concourse.bass as bass
import concourse.tile as tile
from concourse import bacc
from concourse import mybir
from concourse.bass_utils import run_bass_kernel_spmd

N_CORES = 8
T, IN = 1024, 4096
Q, KV = 4096, 1024
OUT = Q + 2 * KV
D = 4
OS = OUT // N_CORES          # 768 output rows per core
OSH = OS // 2                # 384, o-half
NB = IN // 128               # 32 k-blocks
NC_CHUNK = IN // 512         # 8 u16-word chunks of 128 words
N_TILES = T // 128           # 8 token tiles
T_PAD = T                    # no padding needed (T % 128 == 0)
UCHUNK = 512

F16 = mybir.dt.float16
F32 = mybir.dt.float32
U16 = mybir.dt.uint16
I32 = mybir.dt.int32


# ---------------------------------------------------------------------------
# Host-side routing schedule
# ---------------------------------------------------------------------------
def _schedule(indices):
    idx = np.asarray(indices).astype(np.int64)
    assert idx.shape == (T,)
    order = np.argsort(idx, kind="stable").astype(np.int32)
    counts = np.bincount(idx, minlength=D).astype(np.int64)
    return tuple(int(c) for c in counts), order


def _pieces(counts):
    """[(d, c0, c1)] column pieces (<=512 wide) in sorted-token space."""
    pieces = []
    c = 0
    for d in range(D):
        n = counts[d]
        if n == 0:
            continue
        p0 = c
        while p0 < c + n:
            p1 = min(p0 + 512, c + n)
            pieces.append((d, p0, p1))
            p0 = p1
        c += n
    return pieces


# ---------------------------------------------------------------------------
# Device program
# ---------------------------------------------------------------------------
def _build_program(counts):
    pieces = _pieces(counts)
    adapters = [d for d in range(D) if counts[d] > 0]

    nc = bacc.Bacc(
        trn_type="TRN2", target_bir_lowering=False, debug=False, num_devices=1
    )
    x_d = nc.dram_tensor("x", [T, IN], F32, kind="ExternalInput").ap()
    gidx_d = nc.dram_tensor("gidx", [128, N_TILES], I32, kind="ExternalInput").ap()
    wbT_d = nc.dram_tensor("wbT", [IN, OS], F16, kind="ExternalInput").ap()
    qwT_d = nc.dram_tensor(
        "qwT", [D, 2, IN // 4, OSH], U16, kind="ExternalInput"
    ).ap()
    scq_d = nc.dram_tensor(
        "scq", [D, 2, 128, 4 * OSH], F16, kind="ExternalInput"
    ).ap()
    auxw_d = nc.dram_tensor("auxw", [3, D * OS], F16, kind="ExternalInput").ap()
    outT_d = nc.dram_tensor("outT", [OS, T_PAD], F32, kind="ExternalOutput").ap()

    with tile.TileContext(nc) as tc:
        with (
            tc.tile_pool(name="misc", bufs=1) as pmisc,
            tc.tile_pool(name="big", bufs=1) as pbig,
            tc.tile_pool(name="gxp", bufs=2) as pgx,
            tc.tile_pool(name="qtp", bufs=2) as pqt,
            tc.tile_pool(name="scp", bufs=2) as psc,
            tc.tile_pool(name="axp", bufs=2) as pax,
            tc.tile_pool(name="oop", bufs=2) as poo,
            tc.tile_pool(name="urp", bufs=1) as pur,
            tc.tile_pool(name="pp", bufs=6, space="PSUM") as pps,
            tc.tile_pool(name="pu", bufs=2, space="PSUM") as ppu,
        ):
            # ---- constants / persistent tiles
            gidx = pmisc.tile([128, N_TILES], I32, tag="gidx")
            nc.gpsimd.dma_start(gidx[:], gidx_d[:])
            wb = pbig.tile([128, NB * OS], F16, tag="wb")
            # wb[p, kb*OS + o] = wbT[kb*128 + p, o] — issued on the scalar
            # queue so the gpsimd queue starts the token gathers immediately
            nc.scalar.dma_start(
                wb[:].rearrange("p (kb o) -> p kb o", kb=NB),
                wbT_d[:].rearrange("(kb p) o -> p kb o", p=128),
            )
            ones_col = pmisc.tile([128, 1], F16, tag="onesc")
            nc.vector.memset(ones_col[:], 1.0)
            aux_x = pmisc.tile([3, T_PAD], F16, tag="auxx")
            nc.vector.memset(aux_x[0:1, :], 1.0)

            xgT = pbig.tile([128, NB * T_PAD], F16, tag="xgT")
            xgT3 = xgT[:].rearrange("p (kb t) -> p kb t", kb=NB)

            def emit_phase_a():
                # gather + batched transpose per token tile
                for ti in range(N_TILES):
                    gx = pgx.tile([128, IN], F16, tag="gx")
                    nc.gpsimd.indirect_dma_start(
                        out=gx[:],
                        out_offset=None,
                        in_=x_d[:],
                        in_offset=bass.IndirectOffsetOnAxis(
                            ap=gidx[:, ti : ti + 1], axis=0
                        ),
                    )
                    # NB: all transposes must go on ONE queue — concurrent
                    # DMA-transposes from two queues clobber shared XBAR state.
                    nc.sync.dma_start(
                        xgT3[:, :, ti * 128 : (ti + 1) * 128],
                        gx[:],
                        transpose=True,
                    )

            # ---- per-adapter merged-weight build (o-halves), then matmuls.
            # wfh[h][p, kb*OSH + o] = w_base + sc*w4 for out col o of half h.
            # Build: (1) nibble-extract straight into the f16 tile as exact
            # subnormal bit patterns n*2^-18 (u32 lanes process 2 words/op),
            # (2) in-place all-f16 multiply by sc*2^18 per C-quad (2x DVE),
            # (3) in-place all-f16 add of w_base (DVE + 2 GpSimd per half).
            wfh = [
                pbig.tile([128, NB * OSH], F16, tag=f"wfh{h}", name=f"wfh{h}")
                for h in range(2)
            ]
            U32 = mybir.dt.uint32
            SHIFTS = {
                0: (mybir.AluOpType.logical_shift_left, 6),
                1: (mybir.AluOpType.logical_shift_left, 2),
                2: (mybir.AluOpType.logical_shift_right, 2),
                3: (mybir.AluOpType.logical_shift_right, 6),
            }
            wb3 = wb[:].rearrange("p (kb o) -> p kb o", kb=NB)

            def emit_u_chunk(ci):
                c0 = ci * UCHUNK
                c1 = min(c0 + UCHUNK, T_PAD)
                clen = c1 - c0
                up = ppu.tile([1, UCHUNK], F32, space="PSUM", tag="up")
                for kb in range(NB):
                    nc.tensor.matmul(
                        up[:, 0:clen],
                        lhsT=ones_col[:],
                        rhs=xgT[:, kb * T_PAD + c0 : kb * T_PAD + c1],
                        start=(kb == 0),
                        stop=(kb == NB - 1),
                    )
                ur = pur.tile([1, UCHUNK], F16, tag="ur")
                url = pur.tile([1, UCHUNK], F16, tag="url")
                nc.vector.tensor_copy(ur[:, 0:clen], up[:, 0:clen])
                nc.vector.tensor_tensor(
                    out=url[:, 0:clen],
                    in0=up[:, 0:clen],
                    in1=ur[:, 0:clen],
                    op=mybir.AluOpType.subtract,
                )
                nc.sync.dma_start(aux_x[1:2, c0:c1], ur[:, 0:clen])
                nc.sync.dma_start(aux_x[2:3, c0:c1], url[:, 0:clen])

            def emit_loads(d, h):
                qth = pqt.tile([128, NC_CHUNK * OSH], U16, tag="qth")
                nc.gpsimd.dma_start(
                    qth[:].rearrange("p (C o) -> p C o", C=NC_CHUNK),
                    qwT_d[d, h].rearrange("(C p) o -> p C o", p=128),
                )
                scq = psc.tile([128, 4 * OSH], F16, tag="scq")
                nc.gpsimd.dma_start(scq[:], scq_d[d, h])
                return qth, scq

            def emit_build(d, h, loads=None):
                qth, scq = loads if loads is not None else emit_loads(d, h)
                for C in range(NC_CHUNK):
                    for jj in range(4):
                        kb = 4 * C + jj
                        op0, sh = SHIFTS[jj]
                        nc.vector.tensor_scalar(
                            out=wfh[h][:, kb * OSH : (kb + 1) * OSH].bitcast(
                                U32
                            ),
                            in0=qth[:, C * OSH : (C + 1) * OSH].bitcast(U32),
                            scalar1=sh,
                            scalar2=0x03C003C0,
                            op0=op0,
                            op1=mybir.AluOpType.bitwise_and,
                        )
                for C in range(NC_CHUNK):
                    quad = wfh[h][:, 4 * C * OSH : (4 * C + 4) * OSH]
                    nc.vector.tensor_tensor(
                        out=quad, in0=quad, in1=scq[:],
                        op=mybir.AluOpType.mult,
                    )
                    nc.vector.tensor_tensor(
                        out=quad,
                        in0=quad,
                        in1=wb3[:, 4 * C : 4 * C + 4, h * OSH : (h + 1) * OSH],
                        op=mybir.AluOpType.add,
                    )

            def emit_te_mains(d, h):
                dp = [p for p in pieces if p[0] == d]
                groups = []
                for lot in range(3):
                    pss = []
                    for _ in dp:
                        ps = pps.tile([128, 512], F32, space="PSUM", tag="pp")
                        pss.append(ps)
                    for kb in range(NB):
                        for pi, (_, pc0, pc1) in enumerate(dp):
                            nc.tensor.matmul(
                                pss[pi][:, 0 : pc1 - pc0],
                                lhsT=wfh[h][
                                    :,
                                    kb * OSH + lot * 128 : kb * OSH
                                    + (lot + 1) * 128,
                                ],
                                rhs=xgT[:, kb * T_PAD + pc0 : kb * T_PAD + pc1],
                                start=(kb == 0),
                                stop=False,
                            )
                    groups.append(pss)
                return groups

            def emit_te_aux(d, h, auxw, groups):
                dp = [p for p in pieces if p[0] == d]
                for lot in range(3):
                    ot = 3 * h + lot
                    pss = groups[lot]
                    for pi, (_, pc0, pc1) in enumerate(dp):
                        plen = pc1 - pc0
                        nc.tensor.matmul(
                            pss[pi][:, 0:plen],
                            lhsT=auxw[0:3, ot * 128 : (ot + 1) * 128],
                            rhs=aux_x[0:3, pc0:pc1],
                            start=False,
                            stop=True,
                        )
                        oo = poo.tile([128, 512], F32, tag="oo")
                        nc.scalar.copy(oo[:, 0:plen], pss[pi][:, 0:plen])
                        nc.scalar.dma_start(
                            outT_d[ot * 128 : (ot + 1) * 128, pc0:pc1],
                            oo[:, 0:plen],
                        )

            def load_auxw(d):
                auxw = pax.tile([3, OS], F16, tag="auxw")
                nc.gpsimd.dma_start(auxw[:], auxw_d[:, d * OS : (d + 1) * OS])
                return auxw

            # Schedule: d0's qth/scq loads go on the gpsimd queue BEFORE the
            # token gathers (in-order queue — otherwise they sit behind
            # gathers that are themselves WAR-blocked on late transposes).
            # d0's mains run before the u colsums (TensorE starts as soon as
            # the first transposes land); d0's aux matmuls are deferred until
            # ur/url exist. Later adapters run normally, each half's build
            # overlapping the previous half's matmuls.
            d0 = adapters[0]
            l00 = emit_loads(d0, 0)
            l01 = emit_loads(d0, 1)
            auxw0 = load_auxw(d0)
            emit_phase_a()
            emit_build(d0, 0, l00)
            g00 = emit_te_mains(d0, 0)
            emit_build(d0, 1, l01)
            g01 = emit_te_mains(d0, 1)
            emit_u_chunk(0)
            emit_u_chunk(1)
            emit_te_aux(d0, 0, auxw0, g00)
            emit_te_aux(d0, 1, auxw0, g01)
            for d in adapters[1:]:
                auxw = load_auxw(d)
                for h in range(2):
                    emit_build(d, h)
                    g = emit_te_mains(d, h)
                    emit_te_aux(d, h, auxw, g)
    nc.compile()
    return nc


# ---------------------------------------------------------------------------
# Host-side data prep
# ---------------------------------------------------------------------------
def _unpack_zeros(qz, o_count):
    o = np.arange(o_count)
    words = qz[:, o >> 3, 0].astype(np.int64)
    return ((words >> (4 * (o & 7))) & 0xF).astype(np.float32)


def _repack_qw(qw_c):
    """[D, OS, IN//8] int32 -> [D, 2, IN//4, OSH] u16 such that extracting
    nibble-plane jj (shift 4*jj) from word row 128*C+p of half h gives the
    f16 weight for k = 512*C + 128*jj + p, output col o (within half h)."""
    D_, O_, KW = qw_c.shape
    w = qw_c.view(np.uint32)
    shifts8 = (4 * np.arange(8, dtype=np.uint32)).reshape(1, 1, 1, 8)
    nib = ((w[:, :, :, None] >> shifts8) & 0xF).astype(np.uint16)  # [D,O,KW,8]
    nib = nib.reshape(D_, O_, KW * 8)  # k = kw*8 + j
    v = nib.reshape(D_, O_, NC_CHUNK, 4, 128)  # [d, o, C, jj, p]
    shifts4 = (4 * np.arange(4, dtype=np.uint16)).reshape(1, 1, 1, 4, 1)
    words = (
        (v.astype(np.uint32) << shifts4.astype(np.uint32)).sum(axis=3) & 0xFFFF
    ).astype(np.uint16)  # [d, o, C, p]
    words = words.transpose(0, 2, 3, 1).reshape(D_, IN // 4, O_)  # [(C,p), o]
    halves = np.stack([words[:, :, :OSH], words[:, :, OSH:]], axis=1)
    return np.ascontiguousarray(halves)


_prog_cache = {}


def kernel(**inputs):
    x = np.ascontiguousarray(np.asarray(inputs["x"], dtype=np.float32))
    w_base = np.asarray(inputs["w_base"], dtype=np.float32)
    bias = np.asarray(inputs["bias"], dtype=np.float32)
    qw_q = np.asarray(inputs["qweight_q"], dtype=np.int32)
    qw_k = np.asarray(inputs["qweight_k"], dtype=np.int32)
    qw_v = np.asarray(inputs["qweight_v"], dtype=np.int32)
    qz_q = np.asarray(inputs["qzeros_q"], dtype=np.int32)
    qz_k = np.asarray(inputs["qzeros_k"], dtype=np.int32)
    qz_v = np.asarray(inputs["qzeros_v"], dtype=np.int32)
    sc_q = np.asarray(inputs["scales_q"], dtype=np.float32)
    sc_k = np.asarray(inputs["scales_k"], dtype=np.float32)
    sc_v = np.asarray(inputs["scales_v"], dtype=np.float32)
    indices = np.asarray(inputs["indices"])

    counts, order = _schedule(indices)

    if counts not in _prog_cache:
        _prog_cache[counts] = _build_program(counts)
    nc = _prog_cache[counts]

    z_q = _unpack_zeros(qz_q, Q)
    z_k = _unpack_zeros(qz_k, KV)
    z_v = _unpack_zeros(qz_v, KV)

    gidx_host = np.ascontiguousarray(order.reshape(N_TILES, 128).T)

    SQ, SK = Q // N_CORES, KV // N_CORES
    in_maps = []
    for c in range(N_CORES):
        qs = slice(SQ * c, SQ * (c + 1))
        ks = slice(SK * c, SK * (c + 1))
        wb = np.concatenate(
            [w_base[qs], w_base[Q + SK * c : Q + SK * (c + 1)],
             w_base[Q + KV + SK * c : Q + KV + SK * (c + 1)]], axis=0
        )  # [OS, IN]
        qw = np.concatenate([qw_q[:, qs], qw_k[:, ks], qw_v[:, ks]], axis=1)
        z = np.concatenate([z_q[:, qs], z_k[:, ks], z_v[:, ks]], axis=1)
        sc = np.concatenate(
            [sc_q[:, qs, 0], sc_k[:, ks, 0], sc_v[:, ks, 0]], axis=1
        )  # [D, OS]
        b = np.concatenate(
            [bias[qs], bias[Q + SK * c : Q + SK * (c + 1)],
             bias[Q + KV + SK * c : Q + KV + SK * (c + 1)]]
        )  # [OS]

        wbT = np.ascontiguousarray(wb.T.astype(np.float16))  # [IN, OS]
        qwT = _repack_qw(np.ascontiguousarray(qw))
        # scq[d, h, p, q*OSH + o] = sc[d, h*OSH + o] * 2^18 (per C-quad mult)
        s18 = (sc * float(2.0**18)).astype(np.float16)  # [D, OS]
        scq = np.empty([D, 2, 128, 4 * OSH], np.float16)
        for d in range(D):
            for h in range(2):
                row = np.tile(s18[d, h * OSH : (h + 1) * OSH], 4)
                scq[d, h] = row[None, :]
        scq = np.ascontiguousarray(scq)
        znr2 = (-(z + 1.0) * sc).astype(np.float16)  # [D, OS]
        auxw = np.zeros([3, D * OS], np.float16)
        auxw[0] = np.tile(b.astype(np.float16), D)
        auxw[1] = znr2.reshape(-1)
        auxw[2] = znr2.reshape(-1)

        in_maps.append(
            {
                "x": x,
                "gidx": gidx_host,
                "wbT": wbT,
                "qwT": qwT,
                "scq": scq,
                "auxw": np.ascontiguousarray(auxw),
            }
        )

    import os

    trace = bool(int(os.environ.get("KERNEL_TRACE", "0")))
    res = run_bass_kernel_spmd(
        nc, in_maps, core_ids=list(range(N_CORES)), trace=trace
    )
    kernel._last_results = res

    out = np.zeros([T, OUT], np.float32)
    for c in range(N_CORES):
        rT = np.asarray(res.results[c]["outT"])  # [OS, T_PAD]
        r = rT.T  # [T_PAD, OS]
        cols = np.concatenate(
            [
                np.arange(SQ * c, SQ * (c + 1)),
                np.arange(Q + SK * c, Q + SK * (c + 1)),
                np.arange(Q + KV + SK * c, Q + KV + SK * (c + 1)),
            ]
        )
        out[order[:, None], cols[None, :]] = r
    return out
